# revision 6
# baseline (speedup 1.0000x reference)
"""Trainium2 Bass kernel for nn_AttentionModule (v2).

Computation (per batch row b, input feature i):
    E      = tanh(x @ E_W + E_b)                      # [B, 50]
    s      = einsum('be,iea->bia', E, A_W) + A_b      # [B, 66, 20]
    A      = softmax(s, -1)[..., 1]                   # [B, 66]
    out    = x * A

Math rewrite (as v1): softmax(s)[1] = 1 / (1 + sum_{a!=1} exp(s_a - s_1)).
Weights are pre-differenced vs column a=1 on the host (the a=1 column is then
identically zero and dropped -> 19 kept columns), the A_b bias is folded into
the matmul via a constant-1 row of E (tanh(0*x + 30) == 1.0).

v2 changes vs the 447us v1 baseline (all engines were jammed at ~440us:
PE@1.2GHz p-state 446us, DVE tensor_reduce 435us, ACT exp 368us):
  - x is ALSO uploaded pre-transposed (xT [66, B]) so mm1 consumes it
    directly: kills 4 PE transposes/macro (fp32, 2cyc/row) and the DVE
    PSUM->SBUF copy. x row-major is still uploaded for the final multiply.
  - mm1 in f32r (1 cyc/row vs 4 for fp32); one [102, 512] PSUM tile holds
    ET for 2 macros (two matmuls at partition offsets 0/51) -> single tanh.
  - a=1 column dropped: mm2/exp width 1254 instead of 1320.
  - exp writes bf16; the grouped 19-column reduce is a pairwise fold TREE of
    tensor_tensor adds (bf16 packed => DVE '2x_1p' mode, 2 elem/cycle) --
    InstTensorReduce has NO fast modes, so the tree is ~1.9x cheaper.
  - the "+1" rides a fused scalar_tensor_tensor; reciprocal via the
    custom-DVE RECIPROCAL_APPROX_FAST (1 cyc/elem vs ~6 for the iterative
    nc.vector.reciprocal).
  - final multiply on the Pool/GPSIMD engine (contiguous 2-input mult).

Predicted busy per 512-row macro: ACT 5.2us (exp 4*1230ns + tanh 306ns) is
the bottleneck; PE 5528cyc (4.6us @1.2GHz p-state, 2.3us @2.4GHz), DVE
~3.5us, Pool ~0.6us, DMA ~2.4us across 16 engines.
"""

import numpy as np

B_TOTAL, INPUT, E_NODE, A_NODE = 262144, 66, 50, 20
N_CORES = 8
B_LOCAL = B_TOTAL // N_CORES          # 32768
NBLK = 4                              # 128-row blocks per macro tile
MACRO = 128 * NBLK                    # 512
DM = 2                                # macros per group (DMA + mm1 batch)
GROUP = DM * MACRO                    # 1024 rows
CONST_ROW_BIAS = 30.0                 # tanh(30) == 1.0 in fp32

A_RED = A_NODE - 1                    # 19 kept softmax columns
NIA = INPUT * A_RED                   # 1254
CHUNK = NIA // 3                      # 418 (per PSUM bank)
NG = INPUT * NBLK                     # 264 groups per macro in the tail

DMA_MACROS = DM                       # kept for test.py --small sizing

MUL_ON = "pool"                       # engine for the final x*rec multiply
EXP_DT = "bf16"                       # exp output dtype ("bf16"|"f32")
RECIP = "fast"                        # "fast" (custom DVE) | "exact"

_CACHE = {}


def _build_bass(n_rows, repeat=1):
    import concourse.bass as bass
    import concourse.bacc as bacc
    import concourse.tile as tile
    from concourse import mybir
    from contextlib import ExitStack

    f32 = mybir.dt.float32
    f32r = mybir.dt.float32r
    bf16 = mybir.dt.bfloat16
    exp_dt = bf16 if EXP_DT == "bf16" else f32
    n_groups = n_rows // GROUP
    assert n_rows % GROUP == 0

    nc = bacc.Bacc("TRN2", target_bir_lowering=False, debug=False,
                   num_devices=N_CORES)

    x_d = nc.dram_tensor("x", [n_rows, INPUT], f32, kind="ExternalInput").ap()
    xt_d = nc.dram_tensor("xT", [INPUT, n_rows], f32r,
                          kind="ExternalInput").ap()
    w1_d = nc.dram_tensor("W1", [INPUT, E_NODE + 1], f32r,
                          kind="ExternalInput").ap()
    b1_d = nc.dram_tensor("b1", [E_NODE + 1, 1], f32,
                          kind="ExternalInput").ap()
    w2_d = nc.dram_tensor("W2", [E_NODE + 1, NIA], f32r,
                          kind="ExternalInput").ap()
    y_d = nc.dram_tensor("y", [n_rows, INPUT], f32, kind="ExternalOutput").ap()

    x_r = x_d.rearrange("(m p) f -> m p f", p=128)
    y_r = y_d.rearrange("(m p) f -> m p f", p=128)

    with tile.TileContext(nc) as tc, ExitStack() as ctx:
        const = ctx.enter_context(tc.tile_pool(name="const", bufs=1))
        xtp = ctx.enter_context(tc.tile_pool(name="xtp", bufs=3))
        xp = ctx.enter_context(tc.tile_pool(name="xp", bufs=3))
        etp = ctx.enter_context(tc.tile_pool(name="etp", bufs=2))
        expp = ctx.enter_context(tc.tile_pool(name="expp", bufs=2))
        wkp = ctx.enter_context(tc.tile_pool(name="wkp", bufs=2))
        dnp = ctx.enter_context(tc.tile_pool(name="dnp", bufs=6))
        outp = ctx.enter_context(tc.tile_pool(name="outp", bufs=2))
        ps_et = ctx.enter_context(tc.tile_pool(name="ps_et", bufs=1,
                                               space="PSUM"))
        ps_s = ctx.enter_context(tc.tile_pool(name="ps_s", bufs=2,
                                              space="PSUM"))

        w1_sb = const.tile([INPUT, E_NODE + 1], f32r)
        nc.sync.dma_start(out=w1_sb, in_=w1_d)
        b1_sb = const.tile([E_NODE + 1, 1], f32)
        nc.sync.dma_start(out=b1_sb, in_=b1_d)
        w2_sb = const.tile([E_NODE + 1, NIA], f32r)
        nc.sync.dma_start(out=w2_sb, in_=w2_d)

        iters = [g for _ in range(repeat) for g in range(n_groups)]

        def emit_front(it):
            """DMA loads + mm1 + tanh for one 1024-row group."""
            g = iters[it]
            xgt = xtp.tile([INPUT, GROUP], f32r, name="xgt")
            nc.sync.dma_start(out=xgt, in_=xt_d[:, g * GROUP:(g + 1) * GROUP])
            xg = xp.tile([128, DM * NBLK, INPUT], f32, name="xg")
            nc.sync.dma_start(
                out=xg,
                in_=x_r[g * DM * NBLK:(g + 1) * DM * NBLK]
                .rearrange("m p f -> p m f"),
            )
            # one [51, 1024] PSUM tile (2 banks) holds ET for both macros;
            # each matmul output stays within one bank; single tanh over both
            et_ps = ps_et.tile([E_NODE + 1, GROUP], f32)
            for m_off in range(DM):
                nc.tensor.matmul(et_ps[:, m_off * MACRO:(m_off + 1) * MACRO],
                                 w1_sb,
                                 xgt[:, m_off * MACRO:(m_off + 1) * MACRO],
                                 start=True, stop=True)
            et_sb = etp.tile([E_NODE + 1, GROUP], f32r)
            nc.scalar.activation(
                et_sb, et_ps, mybir.ActivationFunctionType.Tanh,
                bias=b1_sb, scale=1.0,
            )
            return xg, et_sb

        add = mybir.AluOpType.add
        mult = mybir.AluOpType.mult
        fronts = {0: emit_front(0)}
        for it in range(len(iters)):
            if it + 1 < len(iters):
                fronts[it + 1] = emit_front(it + 1)
            xg, et_sb = fronts.pop(it)
            og = outp.tile([128, DM * NBLK, INPUT], f32, name="og")

            for m_off in range(DM):
                exp_g = expp.tile([128, NBLK * NIA], exp_dt, name="exp_g")
                for b in range(NBLK):
                    s_ps = ps_s.tile([128, 3 * 512], f32)
                    lhs = et_sb[:, m_off * MACRO + b * 128:
                                m_off * MACRO + (b + 1) * 128]
                    for c in range(3):
                        nc.tensor.matmul(
                            s_ps[:, c * 512:c * 512 + CHUNK], lhs,
                            w2_sb[:, c * CHUNK:(c + 1) * CHUNK],
                            start=True, stop=True,
                        )
                    exp_sb = exp_g[:, b * NIA:(b + 1) * NIA]
                    nc.scalar.activation(
                        exp_sb.rearrange("p (c w) -> p c w", w=CHUNK),
                        s_ps.rearrange("p (c w) -> p c w", w=512)[:, :, 0:CHUNK],
                        mybir.ActivationFunctionType.Exp,
                    )

                # tail: den = 1 + sum of the 19 kept columns, via a bf16
                # pairwise fold tree (tensor_tensor add gets the DVE 2x_1p
                # packed mode; tensor_reduce has no fast mode at all).
                E3 = exp_g.rearrange("p (g a) -> p g a", a=A_RED)  # [128,264,19]
                wk = wkp.tile([128, NG * 9], exp_dt, name="wk")
                W3 = wk.rearrange("p (g a) -> p g a", a=9)
                nc.vector.tensor_tensor(out=W3[:, :, 0:9], in0=E3[:, :, 0:9],
                                        in1=E3[:, :, 9:18], op=add)
                nc.vector.tensor_tensor(out=W3[:, :, 0:4], in0=W3[:, :, 0:4],
                                        in1=W3[:, :, 4:8], op=add)
                nc.vector.tensor_tensor(out=W3[:, :, 0:2], in0=W3[:, :, 0:2],
                                        in1=W3[:, :, 2:4], op=add)
                nc.vector.tensor_tensor(out=W3[:, :, 0:1], in0=W3[:, :, 0:1],
                                        in1=W3[:, :, 1:2], op=add)
                vt = dnp.tile([128, NG], f32, name="vt")
                V3 = vt.rearrange("p (g a) -> p g a", a=1)
                # V = (w8 + 1) + e18   (the dropped a=1 column contributes
                # exp(0)=1; fused so no separate +1 op)
                nc.vector.scalar_tensor_tensor(
                    out=V3, in0=W3[:, :, 8:9], scalar=1.0,
                    in1=E3[:, :, 18:19], op0=add, op1=add,
                )
                den = dnp.tile([128, NG], f32, name="den")
                D3 = den.rearrange("p (g a) -> p g a", a=1)
                nc.vector.tensor_tensor(out=D3, in0=W3[:, :, 0:1], in1=V3,
                                        op=add)
                rec = dnp.tile([128, NG], f32, name="rec")
                if RECIP == "fast":
                    nc.vector.reciprocal_approx_fast(out=rec, in_=den)
                else:
                    nc.vector.reciprocal(out=rec, in_=den)

                xs = xg[:, m_off * NBLK:(m_off + 1) * NBLK, :]
                ys = og[:, m_off * NBLK:(m_off + 1) * NBLK, :]
                rec3 = rec.rearrange("p (t f) -> p t f", f=INPUT)
                if MUL_ON == "pool":
                    nc.gpsimd.tensor_tensor(out=ys, in0=xs, in1=rec3, op=mult)
                else:
                    nc.vector.tensor_tensor(out=ys, in0=xs, in1=rec3, op=mult)

            g = iters[it]
            nc.sync.dma_start(
                out=y_r[g * DM * NBLK:(g + 1) * DM * NBLK]
                .rearrange("m p f -> p m f"),
                in_=og,
            )

    nc.compile()
    return nc


def _prep_weights(E_W, E_b, A_W, A_b):
    E_W = np.asarray(E_W, dtype=np.float32)
    E_b = np.asarray(E_b, dtype=np.float32)
    A_W = np.asarray(A_W, dtype=np.float32)
    A_b = np.asarray(A_b, dtype=np.float32)
    w1 = np.concatenate([E_W, np.zeros((INPUT, 1), np.float32)], axis=1)
    b1 = np.concatenate([E_b, np.float32([CONST_ROW_BIAS])]).reshape(-1, 1)
    dW = A_W - A_W[:, :, 1:2]                        # [66, 50, 20]
    db = A_b - A_b[:, 1:2]                           # [66, 20]
    dW = np.delete(dW, 1, axis=2)                    # [66, 50, 19]
    db = np.delete(db, 1, axis=1)                    # [66, 19]
    w2 = np.concatenate(
        [dW.transpose(1, 0, 2).reshape(E_NODE, NIA),
         db.reshape(1, NIA)], axis=0,
    ).astype(np.float32)                             # [51, 1254]
    return np.ascontiguousarray(w1), np.ascontiguousarray(b1), \
        np.ascontiguousarray(w2)


def _make_in_maps(x, E_W, E_b, A_W, A_b):
    x = np.ascontiguousarray(np.asarray(x, dtype=np.float32))
    n_local = x.shape[0] // N_CORES
    w1, b1, w2 = _prep_weights(E_W, E_b, A_W, A_b)
    in_maps = []
    for i in range(N_CORES):
        xi = x[i * n_local:(i + 1) * n_local]
        in_maps.append({
            "x": xi,
            "xT": np.ascontiguousarray(xi.T),
            "W1": w1, "b1": b1, "W2": w2,
        })
    return in_maps, n_local


def _run(x, E_W, E_b, A_W, A_b, trace=False):
    from concourse.bass_utils import run_bass_kernel_spmd

    in_maps, n_local = _make_in_maps(x, E_W, E_b, A_W, A_b)
    key = ("nc", n_local)
    if key not in _CACHE:
        _CACHE[key] = _build_bass(n_local)
    nc = _CACHE[key]
    res = run_bass_kernel_spmd(nc, in_maps, list(range(N_CORES)), trace=trace)
    out = np.concatenate([res.results[i]["y"] for i in range(N_CORES)], axis=0)
    return out, res


def kernel(x, E_W, E_b, A_W, A_b):
    out, _ = _run(x, E_W, E_b, A_W, A_b, trace=False)
    return out


# revision 26
# speedup vs baseline: 1.8058x; 1.8058x over previous
"""Trainium2 Bass kernel for nn_AttentionModule (v4, ~358us vs 447us v1).

Computation (per batch row b, input feature i):
    E      = tanh(x @ E_W + E_b)                      # [B, 50]
    s      = einsum('be,iea->bia', E, A_W) + A_b      # [B, 66, 20]
    A      = softmax(s, -1)[..., 1]                   # [B, 66]
    out    = x * A

Math rewrite: softmax(s)[1] = 1 / (1 + sum_{a!=1} exp(s_a - s_1)).
Weights are pre-differenced vs column a=1 on the host (that column becomes
identically zero and is dropped -> 19 kept columns); the A_b bias is folded
into the matmul via a constant-1 row of E (tanh(0*x + 30) == 1.0).

Pure data-parallel over 8 cores (32768 rows/core; 64 macros of 512 rows).

Design (engine budget per 512-row macro, sim-validated within 0.1% of HW):
  - x is uploaded BOTH row-major (final multiply) and pre-transposed
    (xT [66, B] feeds mm1 directly): no PE transposes, no PSUM->SBUF copy.
  - mm1 + tanh per macro in f32r (1 cyc/row); two 1-bank ET PSUM tiles so
    the next tanh is ready mid-group (a single tile phase-locks every group
    boundary into a ~1.3us all-engine stall).
  - mm2: 3 f32r matmuls of 418 cols into a [128, 1536] PSUM tile (2-deep).
    Per 418-col chunk the host packs [22 i-groups x 16 "exact" cols |
    22 x 3 "approx" cols].
  - exact cols: one ACT exp per block ([128, 3, 352] strided -> bf16 SBUF).
    ACT is the bottleneck engine (~87% busy: 4x1065ns exp + 612ns tanh).
  - approx cols: Schraudolph exp on the DVE. The matmul itself emits
    t = 2^23*(127 - 0.044 + s*log2e) (scale+bias folded into W2/its bias
    row on the host); ONE fp32->int32 converting tensor_copy writes the
    bit pattern of ~exp(s) (+-3% max, ~2% rms — only ~16% of the softmax
    denominator mass, total L2 err ~2e-3 vs the 2e-2 gate). This moves
    3/19 of the exp stream off the saturated ACT engine.
  - den = 1 + sum of 19: bf16 pairwise fold tree on DVE (tensor_tensor
    add has the 2x_1p packed mode; tensor_reduce has NO fast mode), the
    "+1" fused into a scalar_tensor_tensor, the Schraudolph triple summed
    on Pool, reciprocal via the 1-cyc/elem RECIPROCAL_APPROX_FAST custom
    DVE op, final x*rec multiply on Pool, per-macro y store.
  - Schedule: DMA loads 2 groups ahead; fronts 1 macro ahead; tails
    deferred 1 macro and interleaved between blocks in 4 stages so the
    DVE fold tree (inputs a macro old) never delays the s-tile-freeing
    copies by more than ~700ns (the copies gate mm2 two blocks ahead,
    which gates the next exp — the critical chain).

Knobs below are A/B-tested via the calibrated TimelineSim (v1 sim 446979ns
vs 447394ns measured on HW; v4 sim 358557ns).
"""

import numpy as np

B_TOTAL, INPUT, E_NODE, A_NODE = 262144, 66, 50, 20
N_CORES = 8
B_LOCAL = B_TOTAL // N_CORES          # 32768
NBLK = 4                              # 128-row blocks per macro tile
MACRO = 128 * NBLK                    # 512
DM = 2                                # macros per group (DMA + mm1 batch)
GROUP = DM * MACRO                    # 1024 rows
CONST_ROW_BIAS = 30.0                 # tanh(30) == 1.0 in fp32

A_RED = A_NODE - 1                    # 19 kept softmax columns
N_APPROX = 3                          # columns exp'd via DVE Schraudolph
N_EXACT = A_RED - N_APPROX            # 16 columns exp'd on ACT
NIA = INPUT * A_RED                   # 1254
CHUNK = NIA // 3                      # 418 (per PSUM bank)
IPC = INPUT // 3                      # 22 i-groups per chunk
CH_EX = IPC * N_EXACT                 # 352 exact cols per chunk (front)
CH_AP = IPC * N_APPROX                # 66 approx cols per chunk (back)
BLK_EX = 3 * CH_EX                    # 1056 exact cols per block
BLK_AP = 3 * CH_AP                    # 198 approx cols per block
NG = INPUT * NBLK                     # 264 groups per macro in the tail
SCHRAUD_A = float(2 ** 23) / float(np.log(2))
SCHRAUD_B = (127.0 - 0.0440) * 2 ** 23

DMA_MACROS = DM                       # kept for test.py --small sizing

MUL_ON = "pool"                       # engine for the final x*rec multiply
EXP_DT = "bf16"                       # exp output dtype ("bf16"|"f32")
RECIP = "fast"                        # "fast" (custom DVE) | "exact"
BUFS = {"xtp": 3, "xp": 3, "etp": 3, "expp": 2, "wkp": 2, "dnp": 6,
        "outp": 2, "ixp": 2}
ABLATE = ""                           # "exp1": exp only chunk 0 (timing only)
LATE_FRONT = False                    # emit next front after macro 0

_CACHE = {}


def _build_bass(n_rows, repeat=1):
    import concourse.bass as bass
    import concourse.bacc as bacc
    import concourse.tile as tile
    from concourse import mybir
    from contextlib import ExitStack

    f32 = mybir.dt.float32
    f32r = mybir.dt.float32r
    bf16 = mybir.dt.bfloat16
    i32 = mybir.dt.int32
    exp_dt = bf16 if EXP_DT == "bf16" else f32
    n_groups = n_rows // GROUP
    assert n_rows % GROUP == 0

    nc = bacc.Bacc("TRN2", target_bir_lowering=False, debug=False,
                   num_devices=N_CORES)

    x_d = nc.dram_tensor("x", [n_rows, INPUT], f32, kind="ExternalInput").ap()
    xt_d = nc.dram_tensor("xT", [INPUT, n_rows], f32r,
                          kind="ExternalInput").ap()
    w1_d = nc.dram_tensor("W1", [INPUT, E_NODE + 1], f32r,
                          kind="ExternalInput").ap()
    b1_d = nc.dram_tensor("b1", [E_NODE + 1, 1], f32,
                          kind="ExternalInput").ap()
    w2_d = nc.dram_tensor("W2", [E_NODE + 1, NIA], f32r,
                          kind="ExternalInput").ap()
    y_d = nc.dram_tensor("y", [n_rows, INPUT], f32, kind="ExternalOutput").ap()

    x_r = x_d.rearrange("(m p) f -> m p f", p=128)
    y_r = y_d.rearrange("(m p) f -> m p f", p=128)

    with tile.TileContext(nc) as tc, ExitStack() as ctx:
        const = ctx.enter_context(tc.tile_pool(name="const", bufs=1))
        xtp = ctx.enter_context(tc.tile_pool(name="xtp", bufs=BUFS["xtp"]))
        xp = ctx.enter_context(tc.tile_pool(name="xp", bufs=BUFS["xp"]))
        etp = ctx.enter_context(tc.tile_pool(name="etp", bufs=BUFS["etp"]))
        ixp = ctx.enter_context(tc.tile_pool(name="ixp", bufs=BUFS["ixp"]))
        expp = ctx.enter_context(tc.tile_pool(name="expp", bufs=BUFS["expp"]))
        wkp = ctx.enter_context(tc.tile_pool(name="wkp", bufs=BUFS["wkp"]))
        dnp = ctx.enter_context(tc.tile_pool(name="dnp", bufs=BUFS["dnp"]))
        outp = ctx.enter_context(tc.tile_pool(name="outp", bufs=BUFS["outp"]))
        ps_et = ctx.enter_context(tc.tile_pool(name="ps_et", bufs=2,
                                               space="PSUM"))
        ps_s = ctx.enter_context(tc.tile_pool(name="ps_s", bufs=2,
                                              space="PSUM"))

        w1_sb = const.tile([INPUT, E_NODE + 1], f32r)
        nc.sync.dma_start(out=w1_sb, in_=w1_d)
        b1_sb = const.tile([E_NODE + 1, 1], f32)
        nc.sync.dma_start(out=b1_sb, in_=b1_d)
        w2_sb = const.tile([E_NODE + 1, NIA], f32r)
        nc.sync.dma_start(out=w2_sb, in_=w2_d)

        iters = [g for _ in range(repeat) for g in range(n_groups)]

        def emit_loads(it):
            """DMA loads for one 1024-row group (issued 2 groups ahead so
            mm1 never waits on DMA at a group boundary)."""
            g = iters[it]
            xgt = xtp.tile([INPUT, GROUP], f32r, name="xgt")
            nc.sync.dma_start(out=xgt, in_=xt_d[:, g * GROUP:(g + 1) * GROUP])
            xg = xp.tile([128, DM * NBLK, INPUT], f32, name="xg")
            nc.sync.dma_start(
                out=xg,
                in_=x_r[g * DM * NBLK:(g + 1) * DM * NBLK]
                .rearrange("m p f -> p m f"),
            )
            return xgt, xg

        def emit_front_macro(mi):
            """mm1 + tanh for ONE macro (per-macro granularity: with two
            1-bank ET PSUM tiles the next tanh is ready mid-group and the
            scheduler can slot it into small ACT gaps instead of
            phase-locking every group boundary)."""
            it, m_off = mi // DM, mi % DM
            xgt, xg = loads_of[it]
            et_ps = ps_et.tile([E_NODE + 1, MACRO], f32)
            nc.tensor.matmul(et_ps, w1_sb,
                             xgt[:, m_off * MACRO:(m_off + 1) * MACRO],
                             start=True, stop=True)
            et_sb = etp.tile([E_NODE + 1, MACRO], f32r)
            nc.scalar.activation(
                et_sb, et_ps, mybir.ActivationFunctionType.Tanh,
                bias=b1_sb, scale=1.0,
            )
            return xg, et_sb

        add = mybir.AluOpType.add
        mult = mybir.AluOpType.mult
        n_macros = len(iters) * DM

        def emit_macro(mi, prev):
            """mm2 + exp + schraudolph-copy for macro mi, with the previous
            macro's tail stages interleaved between blocks."""
            it, m_off = mi // DM, mi % DM
            xg, et_sb = front_of[mi]
            exp_g = expp.tile([128, NBLK * BLK_EX], exp_dt, name="exp_g")
            ix_g = ixp.tile([128, NBLK * BLK_AP], i32, name="ix_g")
            tail = _tail_stages(prev) if prev is not None else None
            for b in range(NBLK):
                s_ps = ps_s.tile([128, 3 * 512], f32)
                lhs = et_sb[:, b * 128:(b + 1) * 128]
                for c in range(3):
                    nc.tensor.matmul(
                        s_ps[:, c * 512:c * 512 + CHUNK], lhs,
                        w2_sb[:, c * CHUNK:(c + 1) * CHUNK],
                        start=True, stop=True,
                    )
                s3 = s_ps.rearrange("p (c w) -> p c w", w=512)
                exp_sb = exp_g[:, b * BLK_EX:(b + 1) * BLK_EX]
                nc.scalar.activation(
                    exp_sb.rearrange("p (c w) -> p c w", w=CH_EX),
                    s3[:, :, 0:CH_EX],
                    mybir.ActivationFunctionType.Exp,
                )
                # Schraudolph exp for the approx columns: the matmul already
                # produced t = 2^23*(127-.044 + s*log2e) (scale and magic
                # bias folded into W2 on the host); one DVE fp32->int32
                # converting copy turns t into the bit pattern of ~exp(s).
                # High priority: this copy frees the PSUM s-tile that gates
                # the mm2 two blocks ahead.
                with tc.high_priority():
                    nc.vector.tensor_copy(
                        out=ix_g[:, b * BLK_AP:(b + 1) * BLK_AP]
                        .rearrange("p (c w) -> p c w", w=CH_AP),
                        in_=s3[:, :, CH_EX:CHUNK],
                    )
                if tail is not None:
                    tail[b]()
            return exp_g, ix_g, xg, m_off, it

        def _tail_stages(state):
            """den/recip/multiply/store for a macro, split into 4 stages
            (deferred one macro: inputs are old, so no stage ever stalls)."""
            exp_g, ix_g, xg, m_off, it = state
            E3 = exp_g.rearrange("p (g a) -> p g a", a=N_EXACT)
            X3 = ix_g.bitcast(f32).rearrange("p (g a) -> p g a", a=N_APPROX)
            wk = wkp.tile([128, NG * 8], exp_dt, name="wk")
            W3 = wk.rearrange("p (g a) -> p g a", a=8)
            vt = dnp.tile([128, NG], f32, name="vt")
            V3 = vt.rearrange("p (g a) -> p g a", a=1)
            den = dnp.tile([128, NG], f32, name="den")
            D3 = den.rearrange("p (g a) -> p g a", a=1)
            rec = dnp.tile([128, NG], f32, name="rec")

            def s0():
                # L1 split in half: a DVE op never occupies the engine for
                # more than ~700ns, so the s-tile-freeing copies slot in
                # with bounded lag.
                nc.vector.tensor_tensor(out=W3[:, :, 0:4], in0=E3[:, :, 0:4],
                                        in1=E3[:, :, 8:12], op=add)
                nc.vector.tensor_tensor(out=W3[:, :, 4:8], in0=E3[:, :, 4:8],
                                        in1=E3[:, :, 12:16], op=add)

            def s1():
                nc.vector.tensor_tensor(out=W3[:, :, 0:4], in0=W3[:, :, 0:4],
                                        in1=W3[:, :, 4:8], op=add)
                # V = (x0 + 1) + x1   (the dropped a=1 column contributes
                # exp(0)=1; fused so no separate +1 op)
                nc.vector.scalar_tensor_tensor(
                    out=V3, in0=X3[:, :, 0:1], scalar=1.0,
                    in1=X3[:, :, 1:2], op0=add, op1=add,
                )

            def s2():
                nc.vector.tensor_tensor(out=W3[:, :, 0:2], in0=W3[:, :, 0:2],
                                        in1=W3[:, :, 2:4], op=add)
                nc.gpsimd.tensor_tensor(out=W3[:, :, 0:1], in0=W3[:, :, 0:1],
                                        in1=W3[:, :, 1:2], op=add)
                nc.gpsimd.tensor_tensor(out=V3, in0=V3, in1=X3[:, :, 2:3],
                                        op=add)

            def s3():
                nc.gpsimd.tensor_tensor(out=D3, in0=W3[:, :, 0:1], in1=V3,
                                        op=add)
                if RECIP == "fast":
                    nc.vector.reciprocal_approx_fast(out=rec, in_=den)
                else:
                    nc.vector.reciprocal(out=rec, in_=den)
                og = outp.tile([128, NBLK, INPUT], f32, name="og")
                xs = xg[:, m_off * NBLK:(m_off + 1) * NBLK, :]
                rec3 = rec.rearrange("p (t f) -> p t f", f=INPUT)
                if MUL_ON == "pool":
                    nc.gpsimd.tensor_tensor(out=og, in0=xs, in1=rec3,
                                            op=mult)
                else:
                    nc.vector.tensor_tensor(out=og, in0=xs, in1=rec3,
                                            op=mult)
                g = iters[it]
                m0 = g * DM * NBLK + m_off * NBLK
                nc.sync.dma_start(
                    out=y_r[m0:m0 + NBLK].rearrange("m p f -> p m f"),
                    in_=og,
                )

            return [s0, s1, s2, s3]

        loads_of = {0: emit_loads(0)}
        if len(iters) > 1:
            loads_of[1] = emit_loads(1)
        front_of = {0: emit_front_macro(0)}
        pending = None
        for mi in range(n_macros):
            it = mi // DM
            if mi % DM == 0 and it + 2 < len(iters):
                loads_of[it + 2] = emit_loads(it + 2)
            if mi + 1 < n_macros:
                front_of[mi + 1] = emit_front_macro(mi + 1)
            pending = emit_macro(mi, pending)
            front_of.pop(mi, None)
            if mi % DM == DM - 1 and it > 0:
                loads_of.pop(it - 1, None)
        for stage in _tail_stages(pending):
            stage()

    nc.compile()
    return nc


def _prep_weights(E_W, E_b, A_W, A_b):
    E_W = np.asarray(E_W, dtype=np.float32)
    E_b = np.asarray(E_b, dtype=np.float32)
    A_W = np.asarray(A_W, dtype=np.float32)
    A_b = np.asarray(A_b, dtype=np.float32)
    w1 = np.concatenate([E_W, np.zeros((INPUT, 1), np.float32)], axis=1)
    b1 = np.concatenate([E_b, np.float32([CONST_ROW_BIAS])]).reshape(-1, 1)
    dW = A_W - A_W[:, :, 1:2]                        # [66, 50, 20]
    db = A_b - A_b[:, 1:2]                           # [66, 20]
    dW = np.delete(dW, 1, axis=2)                    # [66, 50, 19]
    db = np.delete(db, 1, axis=1)                    # [66, 19]
    w2f = np.concatenate(
        [dW.transpose(1, 0, 2),
         db.reshape(INPUT, 1, A_RED).transpose(1, 0, 2)], axis=0,
    ).astype(np.float64)                             # [51, 66, 19]
    # Schraudolph folding for the last N_APPROX kept columns of each i:
    # t = s*2^23*log2(e) + 2^23*(127-.044); bias rides the const-1 ET row.
    w2f[:, :, N_EXACT:] *= SCHRAUD_A
    w2f[E_NODE, :, N_EXACT:] += SCHRAUD_B
    # chunk layout: for each chunk of 22 i-groups, exact cols first
    # (22*16) then approx cols (22*3)
    w2c = w2f.reshape(E_NODE + 1, 3, IPC, A_RED)
    w2 = np.concatenate(
        [w2c[:, :, :, :N_EXACT].reshape(E_NODE + 1, 3, CH_EX),
         w2c[:, :, :, N_EXACT:].reshape(E_NODE + 1, 3, CH_AP)], axis=2,
    ).reshape(E_NODE + 1, NIA).astype(np.float32)    # [51, 1254]
    return np.ascontiguousarray(w1), np.ascontiguousarray(b1), \
        np.ascontiguousarray(w2)


def _make_in_maps(x, E_W, E_b, A_W, A_b):
    x = np.ascontiguousarray(np.asarray(x, dtype=np.float32))
    n_local = x.shape[0] // N_CORES
    w1, b1, w2 = _prep_weights(E_W, E_b, A_W, A_b)
    in_maps = []
    for i in range(N_CORES):
        xi = x[i * n_local:(i + 1) * n_local]
        in_maps.append({
            "x": xi,
            "xT": np.ascontiguousarray(xi.T),
            "W1": w1, "b1": b1, "W2": w2,
        })
    return in_maps, n_local


def _run(x, E_W, E_b, A_W, A_b, trace=False):
    from concourse.bass_utils import run_bass_kernel_spmd

    in_maps, n_local = _make_in_maps(x, E_W, E_b, A_W, A_b)
    key = ("nc", n_local)
    if key not in _CACHE:
        _CACHE[key] = _build_bass(n_local)
    nc = _CACHE[key]
    res = run_bass_kernel_spmd(nc, in_maps, list(range(N_CORES)), trace=trace)
    out = np.concatenate([res.results[i]["y"] for i in range(N_CORES)], axis=0)
    return out, res


def kernel(x, E_W, E_b, A_W, A_b):
    out, _ = _run(x, E_W, E_b, A_W, A_b, trace=False)
    return out
